# revision 14
# baseline (speedup 1.0000x reference)
"""Causal self-attention (B=4, T=2048, C=1024, H=16) on 8 Trainium2 cores.

Sharding: core c handles batch b = c // 2 and head group g = c % 2
(heads 8g..8g+7, i.e. a 512-wide slice of the QKV/proj feature dim).
Each core computes q/k/v projections for its slice, causal attention for
its 8 heads, and a partial output projection; the host sums the two
partials per batch (the "all-reduce after proj") and transposes back.

On-chip layout is fully transposed (feature dim on partitions, time on
the free axis) so that attention scores come out as S^T[tk, tq] and can
feed the P@V matmul without any on-chip transposes.  Softmax denominators
ride along as an extra ones-column appended to V (row 64 of the PV psum).
All matmuls run in float32r (TF32-like) which is full PE speed for
moving dims >= 256.
"""

import sys

for _p in ("/root/.axon_site/_ro/trn_rl_repo", "/opt/trn_rl_repo"):
    if _p not in sys.path:
        sys.path.append(_p)

import numpy as np

import concourse.bass as bass
import concourse.mybir as mybir
import concourse.tile as tile
from concourse import bacc
from concourse.bass_utils import run_bass_kernel_spmd

B, T, C, H = 4, 2048, 1024, 16
HD = C // H  # 64 head dim
J = C // 2  # 512: per-core feature slice (8 heads)
P = 128
NCORES = 8
F32 = mybir.dt.float32
F32R = mybir.dt.float32r
BF16 = mybir.dt.bfloat16
import os
MMDT = BF16 if os.environ.get("KERNEL_MMDT", "bf16") == "bf16" else F32R
AF = mybir.ActivationFunctionType

# V'' layout: per head 65 columns (64 v dims + ones); a PV matmul reads a
# 128-wide window starting at h*65 so that psum row 64 is the softmax sum.
VSTRIDE = 65
VFREE = 7 * VSTRIDE + P  # 583
VFREE_PAD = 584

_cache = {}


def _build_nc(trace_scopes=False):
    nc = bacc.Bacc("TRN2", target_bir_lowering=False, debug=False)

    xT = nc.declare_dram_parameter("xT", [C, T], MMDT, isOutput=False)
    wqT = nc.declare_dram_parameter("wqT", [C, J], MMDT, isOutput=False)
    wkT = nc.declare_dram_parameter("wkT", [C, J], MMDT, isOutput=False)
    wvT = nc.declare_dram_parameter("wvT", [C, J], MMDT, isOutput=False)
    wpT = nc.declare_dram_parameter("wpT", [J, C], MMDT, isOutput=False)
    bq2 = nc.declare_dram_parameter("bq2", [P, J // P], F32, isOutput=False)
    bk2 = nc.declare_dram_parameter("bk2", [P, J // P], F32, isOutput=False)
    bpe = nc.declare_dram_parameter("bpe", [P, C // P], F32, isOutput=False)
    maskp = nc.declare_dram_parameter("mask", [P, 512], MMDT, isOutput=False)
    outT = nc.declare_dram_parameter("outT", [C, T], F32, isOutput=True)

    xT_v = xT[:, :].rearrange("(cc p) t -> p cc t", p=P)  # [128, 8, 2048]
    wqT_v = wqT[:, :].rearrange("(cc p) j -> p cc j", p=P)  # [128, 8, 512]
    wkT_v = wkT[:, :].rearrange("(cc p) j -> p cc j", p=P)
    wvT_v = wvT[:, :].rearrange("(cc p) j -> p cc j", p=P)
    wpT_v = wpT[:, :].rearrange("(jc p) e -> p jc e", p=P)  # [128, 4, 1024]
    outT_v = outT[:, :].rearrange("(ec p) t -> p ec t", p=P)  # [128, 8, 2048]

    NTC = T // 512  # 4 time chunks of 512
    NJC = J // P  # 4 feature chunks per core slice
    NCC = C // P  # 8 contraction chunks
    NEC = C // P  # 8 output feature chunks

    with tile.TileContext(nc) as tc:
        with (
            tc.tile_pool(name="persist", bufs=1) as persist,
            tc.tile_pool(name="xstream", bufs=2) as xstream,
            tc.tile_pool(name="proj_out", bufs=4) as proj_out,
            tc.tile_pool(name="ytiles", bufs=2) as ytiles,
            tc.tile_pool(name="ptiles", bufs=6) as ptiles,
            tc.tile_pool(name="ztiles", bufs=4) as ztiles,
            tc.tile_pool(name="psAC", bufs=2, space="PSUM") as psAC,
            tc.tile_pool(name="psS", bufs=2, space="PSUM") as psS,
            tc.tile_pool(name="psY", bufs=2, space="PSUM") as psY,
        ):
            # ---- persistent SBUF tensors -------------------------------
            qT_sb = persist.tile([P, NJC, T], MMDT)  # [128, 4, 2048]
            kT_sb = persist.tile([P, NJC, T], MMDT)
            v_sb = persist.tile([P, T // P, VFREE_PAD], MMDT)  # [128, 16, 584]
            bq_sb = persist.tile([P, NJC], F32)
            bk_sb = persist.tile([P, NJC], F32)
            bpe_sb = persist.tile([P, NEC], F32)
            mask_sb = persist.tile([P, 512], MMDT)
            wq_sb = persist.tile([P, NCC, J], MMDT)
            wk_sb = persist.tile([P, NCC, J], MMDT)
            wv_sb = persist.tile([P, NCC, J], MMDT)
            wp_sb = persist.tile([P, NJC, C], MMDT)  # [128, 4, 1024]

            nc.sync.dma_start(out=wq_sb[:, :4], in_=wqT_v[:, :4])
            nc.sync.dma_start(out=wq_sb[:, 4:], in_=wqT_v[:, 4:])
            nc.sync.dma_start(out=bq_sb, in_=bq2[:, :])
            nc.sync.dma_start(out=bk_sb, in_=bk2[:, :])
            nc.sync.dma_start(out=bpe_sb, in_=bpe[:, :])
            nc.sync.dma_start(out=mask_sb, in_=maskp[:, :])
            # zero the tail of v_sb once (beyond head 7's ones column)
            nc.vector.tensor_copy(
                v_sb[:, :, 8 * VSTRIDE :],
                nc.const_aps.tensor(0.0, [P, T // P, VFREE_PAD - 8 * VSTRIDE], F32),
            )

            def qkv_chunk(tc_i, xt):
                """q/k/v projections for one 512-wide time chunk."""
                ts = slice(tc_i * 512, (tc_i + 1) * 512)
                for jc in range(NJC):
                    jsl = slice(jc * P, (jc + 1) * P)
                    q_ps = psAC.tile([P, 512], F32, tag="psAC", name="q_ps")
                    for cc in range(NCC):
                        nc.tensor.matmul(
                            q_ps[:],
                            wq_sb[:, cc, jsl],
                            xt[:, cc, :],
                            start=(cc == 0),
                            stop=(cc == NCC - 1),
                        )
                    nc.vector.tensor_scalar_add(
                        qT_sb[:, jc, ts], q_ps[:], bq_sb[:, jc : jc + 1]
                    )
                    k_ps = psAC.tile([P, 512], F32, tag="psAC", name="k_ps")
                    for cc in range(NCC):
                        nc.tensor.matmul(
                            k_ps[:],
                            wk_sb[:, cc, jsl],
                            xt[:, cc, :],
                            start=(cc == 0),
                            stop=(cc == NCC - 1),
                        )
                    nc.vector.tensor_scalar_add(
                        kT_sb[:, jc, ts], k_ps[:], bk_sb[:, jc : jc + 1]
                    )
                for s4 in range(4):
                    t16 = tc_i * 4 + s4
                    v_ps = psAC.tile([P, 512], F32, tag="psAC", name="v_ps")
                    for cc in range(NCC):
                        nc.tensor.matmul(
                            v_ps[:],
                            xt[:, cc, s4 * P : (s4 + 1) * P],
                            wv_sb[:, cc, :],
                            start=(cc == 0),
                            stop=(cc == NCC - 1),
                        )
                    vrow = v_sb[:, t16, : 8 * VSTRIDE].rearrange(
                        "p (h d) -> p h d", d=VSTRIDE
                    )
                    nc.vector.tensor_copy(
                        vrow[:, :, :HD],
                        v_ps[:].rearrange("p (h d) -> p h d", d=HD),
                    )
                    nc.vector.tensor_copy(
                        vrow[:, :, HD : HD + 1],
                        nc.const_aps.tensor(1.0, [P, 8, 1], F32),
                    )

            def attention_chunk(qc, filler=None):
                """causal attention for q chunk qc; returns the yt tile.
                filler(ph) emits PE-dense work (prev chunk's projection)
                between head pairs to pad ACT-bound stretches."""
                n_kc = 4 * qc + 4
                yt = ytiles.tile([P, NJC, 512], MMDT, tag="yt", name="yt")
                for ph in range(NJC):  # head pair (2ph, 2ph+1)
                    if filler is not None:
                        filler(ph)
                    y_ps = [
                        psY.tile([P, 512], F32, tag="psY", name="y_ps0"),
                        psY.tile([P, 512], F32, tag="psY", name="y_ps1"),
                    ]

                    def win(kc, qc=qc):
                        r = kc - 4 * qc
                        return (128 * r, 512 - 128 * r) if r >= 0 else (0, 512)

                    # software pipeline: score+exp for kc, PV for kc-1
                    p01s = {}
                    for it in range(n_kc + 1):
                        if it < n_kc:
                            kc = it
                            off, W = win(kc)
                            s01 = psS.tile([P, 2, 512], F32, tag="s01", name="s01")
                            for i in range(2):
                                prt = slice(64 * i, 64 * i + 64)
                                nc.tensor.matmul(
                                    s01[:, i, off : off + W],
                                    kT_sb[prt, ph, kc * P : (kc + 1) * P],
                                    qT_sb[prt, ph, qc * 512 + off : (qc + 1) * 512],
                                    start=True,
                                    stop=True,
                                )
                            p01 = ptiles.tile([P, 2, 512], MMDT, tag="p01", name="p01")
                            nc.scalar.activation(
                                out=p01[:, :, off : off + W],
                                in_=s01[:, :, off : off + W],
                                func=AF.Exp,
                                bias=0.0,
                                scale=float(1.0 / np.sqrt(HD)),
                            )
                            if kc - 4 * qc >= 0:
                                nc.vector.tensor_mul(
                                    p01[:, :, off : off + W],
                                    p01[:, :, off : off + W],
                                    mask_sb[:, None, :W].to_broadcast([P, 2, W]),
                                )
                            p01s[kc] = p01
                        if it >= 1:
                            kc = it - 1
                            off, W = win(kc)
                            p01 = p01s.pop(kc)
                            for i in range(2):
                                h = 2 * ph + i
                                nc.tensor.matmul(
                                    y_ps[i][:, off : off + W],
                                    v_sb[:, kc, h * VSTRIDE : h * VSTRIDE + P],
                                    p01[:, i, off : off + W],
                                    start=(kc == 0),
                                    stop=(kc == n_kc - 1),
                                    skip_group_check=True,
                                )
                    for i in range(2):
                        # row 64 of y psum = softmax denominator
                        zraw = ztiles.tile([1, 512], F32, tag="zraw", name="zraw")
                        nc.vector.tensor_copy(zraw[:], y_ps[i][64:65, :])
                        zrec = ztiles.tile([1, 512], F32, tag="zrec", name="zrec")
                        nc.vector.reciprocal_approx_fast(zrec[:], zraw[:])
                        zb = ztiles.tile([64, 512], F32, tag="zb", name="zb")
                        nc.gpsimd.partition_broadcast(zb[:], zrec[:])
                        nc.vector.tensor_mul(
                            yt[64 * i : 64 * i + 64, ph, :],
                            y_ps[i][0:64, :],
                            zb[:],
                        )
                return yt

            def proj_chunk(qc, yt, ecs=None):
                """output projection for one q chunk."""
                qsl = slice(qc * 512, (qc + 1) * 512)
                for ec in ecs if ecs is not None else range(NEC):
                    o_ps = psAC.tile([P, 512], F32, tag="psAC", name="o_ps")
                    for jc in range(NJC):
                        nc.tensor.matmul(
                            o_ps[:],
                            wp_sb[:, jc, ec * P : (ec + 1) * P],
                            yt[:, jc, :],
                            start=(jc == 0),
                            stop=(jc == NJC - 1),
                        )
                    o_sb = proj_out.tile([P, 512], F32, tag="osb", name="o_sb")
                    nc.scalar.activation(
                        out=o_sb[:],
                        in_=o_ps[:],
                        func=AF.Identity,
                        bias=bpe_sb[:, ec : ec + 1],
                        scale=1.0,
                    )
                    nc.sync.dma_start(out=outT_v[:, ec, qsl], in_=o_sb[:])

            # Interleave: qkv(tc), attention(qc=tc) — attention for q chunk
            # qc only needs k/v through time chunk qc (causality), so the
            # PE-dense qkv work of chunk tc+1 fills the ACT-bound attention
            # of chunk tc.
            yts = {}
            for tc_i in range(NTC):
                ts = slice(tc_i * 512, (tc_i + 1) * 512)
                xt = xstream.tile([P, NCC, 512], MMDT, tag="xt", name="xt")
                if tc_i == 0:
                    # split the first loads so the first matmul group can
                    # start as soon as half the data has landed
                    nc.sync.dma_start(out=xt[:, :4, :], in_=xT_v[:, :4, ts])
                    nc.sync.dma_start(out=xt[:, 4:, :], in_=xT_v[:, 4:, ts])
                    nc.sync.dma_start(out=wk_sb, in_=wkT_v)
                    nc.sync.dma_start(out=wv_sb, in_=wvT_v)
                    nc.sync.dma_start(out=wp_sb, in_=wpT_v)
                else:
                    nc.sync.dma_start(out=xt, in_=xT_v[:, :, ts])
                qkv_chunk(tc_i, xt)
                if tc_i >= 1:
                    prev_yt = yts.pop(tc_i - 1)
                    yts[tc_i] = attention_chunk(
                        tc_i,
                        filler=lambda ph, q=tc_i - 1, y=prev_yt: proj_chunk(
                            q, y, ecs=range(2 * ph, 2 * ph + 2)
                        ),
                    )
                else:
                    yts[tc_i] = attention_chunk(tc_i)
            proj_chunk(NTC - 1, yts.pop(NTC - 1))

    nc.compile()
    return nc


def _get_nc():
    if "nc" not in _cache:
        _cache["nc"] = _build_nc()
    return _cache["nc"]


def _prep_in_maps(x, Wq, bq, Wk, bk, Wv, bv, Wp, bp):
    if MMDT == BF16:
        import ml_dtypes

        mm_np = ml_dtypes.bfloat16
    else:
        mm_np = np.float32
    x = np.ascontiguousarray(np.asarray(x, dtype=np.float32))
    Wq = np.asarray(Wq, dtype=np.float32)
    Wk = np.asarray(Wk, dtype=np.float32)
    Wv = np.asarray(Wv, dtype=np.float32)
    Wp = np.asarray(Wp, dtype=np.float32)
    bq = np.asarray(bq, dtype=np.float32)
    bk = np.asarray(bk, dtype=np.float32)
    bv = np.asarray(bv, dtype=np.float32)
    bp = np.asarray(bp, dtype=np.float32)

    mask = (np.arange(P)[:, None] <= np.arange(512)[None, :]).astype(np.float32)

    in_maps = []
    for c in range(NCORES):
        b, g = c // 2, c % 2
        js = slice(g * J, (g + 1) * J)
        # bv folds into the proj bias: Wp[:, js] @ bv[js]; bp only on g==0.
        bpe = Wp[:, js] @ bv[js]
        if g == 0:
            bpe = bpe + bp
        in_maps.append(
            {
                "xT": np.ascontiguousarray(x[b].T.astype(mm_np)),
                "wqT": np.ascontiguousarray(Wq[js, :].T.astype(mm_np)),
                "wkT": np.ascontiguousarray(Wk[js, :].T.astype(mm_np)),
                "wvT": np.ascontiguousarray(Wv[js, :].T.astype(mm_np)),
                "wpT": np.ascontiguousarray(Wp[:, js].T.astype(mm_np)),
                "bq2": np.ascontiguousarray(bq[js].reshape(J // P, P).T),
                "bk2": np.ascontiguousarray(bk[js].reshape(J // P, P).T),
                "bpe": np.ascontiguousarray(bpe.reshape(C // P, P).T),
                "mask": mask.astype(mm_np),
                "outT": np.zeros((C, T), dtype=np.float32),
            }
        )
    return in_maps


def kernel(x, Wq, bq, Wk, bk, Wv, bv, Wp, bp, _trace=False, _ret_extra=None):
    nc = _get_nc()
    in_maps = _prep_in_maps(x, Wq, bq, Wk, bk, Wv, bv, Wp, bp)
    res = run_bass_kernel_spmd(nc, in_maps, list(range(NCORES)), trace=_trace)
    out = np.empty((B, T, C), dtype=np.float32)
    for b in range(B):
        out[b] = (res.results[2 * b]["outT"] + res.results[2 * b + 1]["outT"]).T
    if _ret_extra is not None:
        _ret_extra["res"] = res
    return out
